# revision 31
# baseline (speedup 1.0000x reference)
"""KMeans assignment kernel (retrieval_knn) for 8 Trainium2 NeuronCores.

Computes argmin_k ||x_n - c_k||^2 for x [262144, 64] f32 against centers
[1024, 64] f32, returning int32 cluster ids [262144].

argmin ||x-c||^2 == argmax s, s = 2x.c - ||c||^2 + C.  Scores come from a
SINGLE fp16 matmul (fp16 keeps 11 mantissa bits; measured end-to-end rel
err ~7e-4): stationary [xhi(64); ones(3)] x moving [fp16(2c^T); n1;n2;n3]
where the n-rows are an fp16 cascade of C - ||c||^2.  ScalarE copies
PSUM->SBUF converting to fp16 (its only full-width pass); DVE runs a
4-level tensor_tensor max tree (fp16 2x mode) for 64 group maxima/tile
and max_index picks the winning group.  Extraction of the position
within the winning group is hybrid, balancing three engines: most
batches spill fp16 scores to DRAM (SP queue) and per-tile [128,1]-offset
indirect DMAs (gpsimd, whose Q7 cores pay ~1us of SWDGE descriptor
generation per gather — HW supports only one offset per partition)
gather each point's winning group for a 16-wide max_index; a few
batches (first at tb=0, while the gather pipeline fills) instead resolve
the full index with a 1024-wide max_index on DVE, shaving the gpsimd
generation load.  The emission order is software-pipelined: each batch's
gather results are consumed TAIL_LAG batches later so the in-order DVE
queue never stalls on an in-flight gather.  id = 16*group + pos in u32.
"""

import numpy as np

N_POINTS = 262144
N_FEATURES = 64
N_CLUSTERS = 1024
N_CORES = 8
PTS_PER_CORE = N_POINTS // N_CORES      # 32768
TILE_P = 128                            # points per tile (partition dim)
N_TILES = PTS_PER_CORE // TILE_P        # 256
C_BIAS = 4.0                            # centers top scores near 0 for fp16

_CACHE = {}


def _build_bass():
    import concourse.bass as bass
    import concourse.bacc as bacc
    import concourse.mybir as mybir
    import concourse.tile as tile
    from contextlib import ExitStack

    f16 = mybir.dt.float16
    f32 = mybir.dt.float32
    u32 = mybir.dt.uint32

    nc = bacc.Bacc(None, target_bir_lowering=False)

    xpack = nc.declare_dram_parameter("xpack", [67, PTS_PER_CORE], f16, isOutput=False)
    cc2 = nc.declare_dram_parameter("cc2", [67, N_CLUSTERS], f16, isOutput=False)
    tg64 = nc.declare_dram_parameter("tg64", [128, 4], u32, isOutput=False)
    out = nc.declare_dram_parameter("out", [128, N_TILES], u32, isOutput=True)

    BT = 4            # tiles per batch (spill granularity)
    G = 64            # groups per tile
    GS = 16           # group size (elements gathered per point)
    XB = 8            # tiles per x load
    MI_SET = {0, 13, 26, 39, 52}  # batches resolved by full-width max_index
                  # (no gather); first at tb=0 where the gather pipeline
                  # is still filling and GpSimd is idle anyway

    # raw DRAM spill buffers (manual multi-buffer, rotating per batch)
    spills = [
        nc.dram_tensor(f"sspill{j}", [128 * BT * G, GS], f16) for j in range(18)
    ]

    with tile.TileContext(nc) as tc, ExitStack() as ctx:
        const_pool = ctx.enter_context(tc.tile_pool(name="const", bufs=1))
        xin_pool = ctx.enter_context(tc.tile_pool(name="xin", bufs=4))
        psum_pool = ctx.enter_context(
            tc.tile_pool(name="psum", bufs=2, space=bass.MemorySpace.PSUM)
        )
        s16_pool = ctx.enter_context(tc.tile_pool(name="s16", bufs=18))
        tree_pool = ctx.enter_context(tc.tile_pool(name="tree", bufs=3))
        small_pool = ctx.enter_context(tc.tile_pool(name="small", bufs=48))
        gv_pool = ctx.enter_context(tc.tile_pool(name="gv", bufs=18))
        rmax_pool = ctx.enter_context(tc.tile_pool(name="rmax", bufs=14))
        out_pool = ctx.enter_context(tc.tile_pool(name="out", bufs=1))

        cc2_t = const_pool.tile([67, N_CLUSTERS], f16)
        nc.sync.dma_start(cc2_t[:], cc2[:])
        tg64_t = const_pool.tile([128, BT], u32)
        nc.sync.dma_start(tg64_t[:], tg64[:])

        outbuf = out_pool.tile([128, N_TILES], u32)

        # software pipeline: batch tb's gather results are consumed (tail)
        # only after batch tb+TAIL_LAG's producers are issued, so the
        # in-order DVE queue never stalls on an in-flight gather
        TAIL_LAG = 16
        pending = []   # (tb, use_gather, m8, gv, gw, jw_src) awaiting tail

        def emit_tail(st):
            ptb, pgather, pm8, pgv, ps16 = st
            jw = small_pool.tile([128, BT, 8], u32)
            if pgather:
                pm8_t, pgw = pm8
                for i in range(BT):
                    nc.vector.max_index(
                        jw[:, i, :],
                        pm8_t[:, i : i + 1].to_broadcast([128, 8]),
                        pgv[:, i, :],
                    )
                # id = 16*gA + j, integer arithmetic on u32
                g16 = small_pool.tile([128, BT], u32)
                nc.vector.tensor_scalar_mul(g16[:], pgw[:, :, 0], GS)
                nc.vector.tensor_tensor(
                    outbuf[:, ptb * BT : (ptb + 1) * BT],
                    g16[:],
                    jw[:, :, 0],
                    op=mybir.AluOpType.add,
                )
            else:
                pm8_t = pm8[0]
                for i in range(BT):
                    nc.vector.max_index(
                        jw[:, i, :],
                        pm8_t[:, i : i + 1].to_broadcast([128, 8]),
                        ps16[:, i, :],
                    )
                nc.vector.tensor_copy(
                    outbuf[:, ptb * BT : (ptb + 1) * BT], jw[:, :, 0]
                )

        for tb in range(N_TILES // BT):
            use_gather = tb not in MI_SET
            s16 = s16_pool.tile([128, BT, N_CLUSTERS], f16)
            rowmaxB = rmax_pool.tile([128, BT, G], f16)
            if use_gather:
                spillb = spills[tb % 18]
                spillb_w = spillb[:].rearrange(
                    "(p i g) e -> p i (g e)", p=128, i=BT
                )
            for i in range(BT):
                t = tb * BT + i
                if t % XB == 0:
                    xst = xin_pool.tile([67, XB, TILE_P], f16)
                    csl = slice(t * TILE_P, (t + XB) * TILE_P)
                    nc.sync.dma_start(
                        xst[:], xpack[:, csl].rearrange("p (b q) -> p b q", b=XB)
                    )
                xi = t % XB
                if i % 2 == 0:
                    ps = psum_pool.tile([128, 2, N_CLUSTERS], f32)
                pi = i % 2
                for kh in range(2):
                    ksl = slice(kh * 512, (kh + 1) * 512)
                    nc.tensor.matmul(
                        ps[:, pi, ksl], xst[:, xi, :], cc2_t[:, ksl],
                        start=True, stop=True,
                    )
                if i % 2 == 1:
                    # ScalarE evacuates 2 tiles of scores PSUM->SBUF as fp16
                    nc.scalar.copy(s16[:, i - 1 : i + 1, :], ps[:])
                    if use_gather:
                        # spill each fp16 pair as soon as it lands (SP queue);
                        # small slabs keep the shared DMA engines free for
                        # the latency-critical gathers
                        nc.sync.dma_start(
                            spillb_w[:, i - 1 : i + 1, :],
                            s16[:, i - 1 : i + 1, :],
                        )
            # 4-level segmented max tree (fp16 2x mode), whole batch per level
            sv = s16[:].rearrange("p i (g e) -> p i g e", g=G)
            t1 = tree_pool.tile([128, BT, G, 8], f16)
            nc.vector.tensor_tensor(
                t1[:], sv[:, :, :, 0:8], sv[:, :, :, 8:16],
                op=mybir.AluOpType.max,
            )
            t2 = tree_pool.tile([128, BT, G, 4], f16)
            nc.vector.tensor_tensor(
                t2[:], t1[:, :, :, 0:4], t1[:, :, :, 4:8],
                op=mybir.AluOpType.max,
            )
            t3 = tree_pool.tile([128, BT, G, 2], f16)
            nc.vector.tensor_tensor(
                t3[:], t2[:, :, :, 0:2], t2[:, :, :, 2:4],
                op=mybir.AluOpType.max,
            )
            nc.vector.tensor_tensor(
                rowmaxB[:],
                t3[:, :, :, 0],
                t3[:, :, :, 1],
                op=mybir.AluOpType.max,
            )
            # per-tile max
            m8 = small_pool.tile([128, BT], f16)
            nc.vector.tensor_reduce(
                m8[:], rowmaxB[:], axis=mybir.AxisListType.X,
                op=mybir.AluOpType.max,
            )
            if use_gather:
                gw = small_pool.tile([128, BT, 8], u32)
                for i in range(BT):
                    nc.vector.max_index(
                        gw[:, i, :],
                        m8[:, i : i + 1].to_broadcast([128, 8]),
                        rowmaxB[:, i, :],
                    )
                # gather row index = p*(BT*G) + i*G + g (tg64: the p,i part)
                offu = small_pool.tile([128, BT], u32)
                nc.vector.tensor_tensor(
                    offu[:], gw[:, :, 0], tg64_t[:], op=mybir.AluOpType.add
                )
                # per-tile [128,1]-offset gathers on the gpsimd queue
                gv = gv_pool.tile([128, BT, GS], f16)
                for i in range(BT):
                    nc.gpsimd.indirect_dma_start(
                        out=gv[:, i, :],
                        out_offset=None,
                        in_=spillb[:],
                        in_offset=bass.IndirectOffsetOnAxis(
                            ap=offu[:, i : i + 1], axis=0
                        ),
                    )
                pending.append((tb, True, (m8, gw), gv, None))
            else:
                pending.append((tb, False, (m8,), None, s16))
            if len(pending) > TAIL_LAG:
                emit_tail(pending.pop(0))

        for st in pending:
            emit_tail(st)

        nc.sync.dma_start(out[:], outbuf[:])

    nc.compile()
    return nc


def _prep(x: np.ndarray, centers: np.ndarray):
    xt = np.ascontiguousarray(x.T)                      # [64, N] f32
    xpack = np.empty((67, xt.shape[1]), np.float16)
    xpack[0:64] = xt.astype(np.float16)
    xpack[64:67] = 1.0                                  # aug rows for the bias

    c2t = np.ascontiguousarray((2.0 * centers).T)       # [64, K] f32
    chi = c2t.astype(np.float16)                        # [64, K] fp16

    # (C - ||c||^2) as a 3-term fp16 cascade on the all-ones stationary rows
    cn = np.sum(centers.astype(np.float64) ** 2, axis=1)
    t = C_BIAS - cn
    n1 = t.astype(np.float16)
    r = t - n1.astype(np.float64)
    n2 = r.astype(np.float16)
    n3 = (r - n2.astype(np.float64)).astype(np.float16)
    cc2 = np.concatenate(
        [chi, n1[None, :], n2[None, :], n3[None, :]], axis=0
    )                                                   # [67, K] fp16

    p = np.arange(128, dtype=np.uint32)[:, None]
    i = np.arange(4, dtype=np.uint32)[None, :]
    tg64 = np.ascontiguousarray(p * np.uint32(4 * 64) + i * np.uint32(64))
    return xpack, cc2, tg64


def kernel(x: np.ndarray, centers: np.ndarray) -> np.ndarray:
    import sys
    if "/opt/trn_rl_repo" not in sys.path:
        sys.path.insert(0, "/opt/trn_rl_repo")
    from concourse.bass_utils import run_bass_kernel_spmd

    x = np.asarray(x, dtype=np.float32)
    centers = np.asarray(centers, dtype=np.float32)

    xpack, cc2, tg64 = _prep(x, centers)

    if "nc" not in _CACHE:
        _CACHE["nc"] = _build_bass()
    nc = _CACHE["nc"]

    in_maps = []
    for c in range(N_CORES):
        sl = slice(c * PTS_PER_CORE, (c + 1) * PTS_PER_CORE)
        in_maps.append(
            {
                "xpack": np.ascontiguousarray(xpack[:, sl]),
                "cc2": cc2,
                "tg64": tg64,
            }
        )

    res = run_bass_kernel_spmd(nc, in_maps, list(range(N_CORES)))

    outs = []
    for c in range(N_CORES):
        o = res.results[c]["out"]                       # [128, N_TILES] uint32
        outs.append(np.asarray(o).astype(np.int64).T.reshape(-1))  # point t*128+p
    ids = np.concatenate(outs)
    return ids.astype(np.int32)


if __name__ == "__main__":
    rng = np.random.default_rng(0)
    x = rng.normal(size=(N_POINTS, N_FEATURES)).astype(np.float32)
    c = rng.normal(size=(N_CLUSTERS, N_FEATURES)).astype(np.float32)
    ids = kernel(x=x, centers=c)
    d = (
        np.sum(x * x, 1)[:, None]
        - 2.0 * (x @ c.T)
        + np.sum(c * c, 1)[None, :]
    )
    ref = np.argmin(np.abs(d), axis=1)
    print("mismatch:", np.mean(ids != ref))
